# revision 46
# baseline (speedup 1.0000x reference)
"""Trainium2 Bass kernel for variable-window left/right max pooling.

out[b, c, t] = max(feat[b, c, max(t-L,0) : t+1]) + max(feat[b, c, t : min(t+R,T)])
with L = max(0, round(reg[b,t,0])), R = clip(round(reg[b,t,1]), 1, T).

Strategy (2 batches/core, pure data parallel over 8 cores, fp16 on device):
  - range-max sparse table st_k[c, x] = max(feat[c, x:x+2^k]), levels 0..5
    (max window len 33), built in [c%128, lev, cb, t(+pad)] layout with 5
    shifted full-width tensor_tensor(max) DVE ops (t-pad columns absorb the
    shifted reads; memset once).
  - levels 1..5 are stored to DRAM scratch (2 chunks) and XBAR-transpose
    loaded into the token table ttab[t%128, t//128, lev, c] (4 merged-level
    transposes per batch, split across the SP and ACT HWDGE sequencers).
    Level 0 comes from a host-transposed featT upload.
  - window maxes: max of 2 table entries; flat indices
    (t//128)*(6*128) + lev*128 + t%128 precomputed host-side from reg;
    4 transposing SBUF-source row-gathers (2KB rows, 256 idx each) per batch
    emit terms in [c%128, cb, t] layout.
  - 2 max + 1 add on DVE; affine store.  Raw Block mode, explicit semaphores.
"""

import sys
import types

import numpy as np


def _install_profile_shim():
    if "antenv.axon_hooks" in sys.modules:
        return
    try:
        hooks = types.ModuleType("antenv.axon_hooks")
        hooks._hook = None
        hooks.set_axon_ntff_profile_hook = lambda h: setattr(hooks, "_hook", h)
        hooks.get_axon_ntff_profile_hook = lambda: hooks._hook
        sys.modules["antenv.axon_hooks"] = hooks
        import antenv

        antenv.axon_hooks = hooks
        from trn_agent_boot.trn_boot import _ntff_profile_via_ctypes

        hooks.set_axon_ntff_profile_hook(
            _ntff_profile_via_ctypes("/opt/axon/libaxon_pjrt.so")
        )
    except Exception:
        pass


_install_profile_shim()

import concourse.bacc as bacc
import concourse.mybir as mybir
from concourse.bass_utils import run_bass_kernel_spmd

B, C, T = 16, 1024, 256
N_CORES = 8
BPC = B // N_CORES
NLEV = 6  # sparse-table levels 0..5
CB = C // 128
NQ = 4 * T
TP = T + 32  # padded t pitch for full-width shifted builds

_LOG2 = np.zeros(65, dtype=np.int32)
for _n in range(1, 65):
    _LOG2[_n] = _n.bit_length() - 1

_CACHE = {}
LAST_RESULT = None


def _build_graph():
    if "nc" in _CACHE:
        return _CACHE["nc"]

    nc = bacc.Bacc("TRN2", target_bir_lowering=False, debug=False,
                   num_devices=N_CORES)
    f16 = mybir.dt.float16
    i16 = mybir.dt.int16

    feat_ext = nc.dram_tensor("feat16", [BPC, C, T], f16,
                              kind="ExternalInput").ap()
    featT_ext = nc.dram_tensor("featT", [BPC, T, C], f16,
                               kind="ExternalInput").ap()
    gidx_ext = nc.dram_tensor("gidx", [BPC, 128, NQ // 16], i16,
                              kind="ExternalInput").ap()
    out_ext = nc.dram_tensor("out", [BPC, C, T], f16,
                             kind="ExternalOutput").ap()

    feat_ct = [nc.alloc_sbuf_tensor(f"feat_ct{b}", [128, CB, TP], f16).ap()
               for b in range(BPC)]
    ctab = [nc.alloc_sbuf_tensor(f"ctab{b}", [128, NLEV - 1, CB, TP],
                                 f16).ap() for b in range(BPC)]
    # token table [t%128, rank=lev*2 + t//128, c]; flat idx = lev*T + t
    ttab = [nc.alloc_sbuf_tensor(f"ttab{b}", [128, 2 * NLEV, C], f16).ap()
            for b in range(BPC)]
    gidx = [nc.alloc_sbuf_tensor(f"gidx{b}", [128, NQ // 16], i16).ap()
            for b in range(BPC)]
    gout = [nc.alloc_sbuf_tensor(f"gout{b}", [128, 4, CB, T], f16).ap()
            for b in range(BPC)]
    lbuf = [nc.alloc_sbuf_tensor(f"lbuf{b}", [128, CB, T], f16).ap()
            for b in range(BPC)]
    obuf = [nc.alloc_sbuf_tensor(f"obuf{b}", [128, CB, T], f16).ap()
            for b in range(BPC)]
    flbuf = [nc.alloc_sbuf_tensor(f"flbuf{b}", [128, 2 * NLEV, 4], f16).ap()
             for b in range(BPC)]
    scratch = [nc.dram_tensor(f"scratch{b}", [NLEV - 1, C, T], f16).ap()
               for b in range(BPC)]

    with nc.Block() as block:
        s_inf = [nc.alloc_semaphore(f"s_inf{b}") for b in range(BPC)]
        s_int = [nc.alloc_semaphore(f"s_int{b}") for b in range(BPC)]
        s_ing = [nc.alloc_semaphore(f"s_ing{b}") for b in range(BPC)]
        s_bld = [nc.alloc_semaphore(f"s_bld{b}") for b in range(BPC)]
        s_sc = [[nc.alloc_semaphore(f"s_sc{b}_{k}") for k in range(NLEV - 1)]
                for b in range(BPC)]
        s_tt = [nc.alloc_semaphore(f"s_tt{b}") for b in range(BPC)]
        s_g = [nc.alloc_semaphore(f"s_g{b}") for b in range(BPC)]
        s_cmb = [nc.alloc_semaphore(f"s_cmb{b}") for b in range(BPC)]
        s_out = [nc.alloc_semaphore(f"s_out{b}") for b in range(BPC)]
        s_fl = [nc.alloc_semaphore(f"s_fl{b}") for b in range(BPC)]

        def emit_stores(eng, b):
            """Per-level scratch stores; s_bld counts 2 memsets + builds."""
            for k in range(1, NLEV):
                eng.wait_ge(s_bld[b], 2 + k)
                eng.dma_start(
                    out=scratch[b][k - 1].rearrange(
                        "(cb p) t -> p cb t", p=128),
                    in_=ctab[b][:, k - 1, :, 0:T],
                ).then_inc(s_sc[b][k - 1], 16)

        def emit_xbars(eng, b):
            # per-(level, tt) [1024, 128] -> [128, 1024] transposes: the one
            # geometry verified bit-exact on hardware.
            for k in range(1, NLEV):
                eng.wait_ge(s_sc[b][k - 1], 16)
                for tt in range(2):
                    eng.dma_start(
                        out=ttab[b][:, 2 * k + tt, :],
                        in_=scratch[b][k - 1][:, tt * 128:(tt + 1) * 128],
                        transpose=True,
                    ).then_inc(s_tt[b], 16)

        @block.sync
        def _(sync):
            sync.dma_start(
                out=feat_ct[0][:, :, 0:T],
                in_=feat_ext[0].rearrange("(cb p) t -> p cb t", p=128),
            ).then_inc(s_inf[0], 16)
            sync.dma_start(
                out=ttab[0][:, 0:2, :],
                in_=featT_ext[0].rearrange("(tt p) c -> p tt c", p=128),
            ).then_inc(s_int[0], 16)
            sync.dma_start(
                out=feat_ct[1][:, :, 0:T],
                in_=feat_ext[1].rearrange("(cb p) t -> p cb t", p=128),
            ).then_inc(s_inf[1], 16)
            sync.dma_start(
                out=ttab[1][:, 0:2, :],
                in_=featT_ext[1].rearrange("(tt p) c -> p tt c", p=128),
            ).then_inc(s_int[1], 16)
            emit_stores(sync, 0)
            emit_stores(sync, 1)
            for b in range(BPC):
                sync.wait_ge(s_cmb[b], 3)
                sync.dma_start(
                    out=out_ext[b].rearrange("(cb p) t -> p cb t", p=128),
                    in_=obuf[b][:, :, :],
                ).then_inc(s_out[b], 16)
            for b in range(BPC):
                sync.wait_ge(s_out[b], 16)

        @block.scalar
        def _(scalar):
            for b in range(BPC):
                scalar.dma_start(out=gidx[b][:, :],
                                 in_=gidx_ext[b]).then_inc(s_ing[b], 16)
            for b in range(BPC):
                emit_xbars(scalar, b)
                # readback after all of this batch's xbar completions: its
                # own completion implies the xbar RX writes are visible
                # before the Pool gathers read ttab.
                scalar.wait_ge(s_tt[b], 16 * 10)
                scalar.dma_start(
                    out=flbuf[b][:, :, :],
                    in_=ttab[b][:, :, 0:4],
                ).then_inc(s_fl[b], 16)

        @block.vector
        def _(vector):
            # interleaved level builds (full width; pad absorbs the shift;
            # pad memsets run on the Pool engine, +2 per b on s_bld)
            for k in range(1, NLEV):
                s = 1 << (k - 1)
                for b in range(BPC):
                    if k == 1:
                        vector.wait_ge(s_inf[b], 16)
                        vector.wait_ge(s_bld[b], 2)
                        src = feat_ct[b][:, :, :]
                    else:
                        vector.wait_ge(s_bld[b], 2 + k - 1)
                        src = ctab[b][:, k - 2, :, :]
                    vector.tensor_tensor(
                        out=ctab[b][:, k - 1, :, 0:T],
                        in0=src[:, :, 0:T],
                        in1=src[:, :, s:s + T],
                        op=mybir.AluOpType.max,
                    ).then_inc(s_bld[b], 1)
            # combines
            for b in range(BPC):
                vector.wait_ge(s_g[b], 64)
                vector.tensor_tensor(
                    out=lbuf[b][:, :, :],
                    in0=gout[b][:, 0, :, :], in1=gout[b][:, 1, :, :],
                    op=mybir.AluOpType.max,
                ).then_inc(s_cmb[b], 1)
                vector.tensor_tensor(
                    out=obuf[b][:, :, :],
                    in0=gout[b][:, 2, :, :], in1=gout[b][:, 3, :, :],
                    op=mybir.AluOpType.max,
                ).then_inc(s_cmb[b], 1)
                vector.wait_ge(s_cmb[b], 2)
                vector.tensor_tensor(
                    out=obuf[b][:, :, :], in0=obuf[b][:, :, :],
                    in1=lbuf[b][:, :, :],
                    op=mybir.AluOpType.add,
                ).then_inc(s_cmb[b], 1)

        @block.gpsimd
        def _(gpsimd):
            for b in range(BPC):
                gpsimd.memset(feat_ct[b][:, :, T:TP], 0.0).then_inc(
                    s_bld[b], 1)
                gpsimd.memset(ctab[b][:, :, :, T:TP], 0.0).then_inc(
                    s_bld[b], 1)
            for b in range(BPC):
                gpsimd.wait_ge(s_ing[b], 16)
                gpsimd.wait_ge(s_int[b], 16)
                gpsimd.wait_ge(s_tt[b], 16 * 10)
                gpsimd.wait_ge(s_fl[b], 16)
                for g in range(4):
                    gpsimd.dma_gather(
                        out_ap=gout[b][:, g, :, :],
                        in_ap=ttab[b].rearrange("p r c -> p (r c)"),
                        idxs_ap=gidx[b][:, g * T // 16:(g + 1) * T // 16],
                        num_idxs=T,
                        num_idxs_reg=T,
                        elem_size=C,
                        transpose=True,
                        queue_num=0,
                        sbuf_tokens_per_rank=128,
                        sbuf_free_dim_per_rank=C * 2,
                    ).then_inc(s_g[b], 16)

    nc.compile()
    _CACHE["nc"] = nc
    return nc


def _host_indices(reg):
    """Flat gather indices [B, 4*T]:
    idx(level, x) = (x//128)*(NLEV*128) + level*128 + (x%128).
    Term order: I[term*T + t] = [left_a, left_b, right_a, right_b]."""
    t = np.arange(T, dtype=np.int64)[None, :]

    def enc(k, x):
        return (2 * k + x // 128) * 128 + (x % 128)

    rl = np.maximum(np.round(reg[:, :, 0]).astype(np.int64), 0)
    l_left = np.maximum(t - rl, 0)
    len_l = t + 1 - l_left
    k_l = _LOG2[np.minimum(len_l, 64)]
    if (len_l > 64).any():
        k_l = np.floor(np.log2(len_l)).astype(np.int64)
    p_l = (1 << k_l).astype(np.int64)
    ia_l = enc(k_l, l_left)
    ib_l = enc(k_l, t + 1 - p_l)

    rr = np.clip(np.round(reg[:, :, 1]).astype(np.int64), 1, T)
    r_right = np.minimum(t + rr, T)
    len_r = r_right - t
    k_r = _LOG2[np.minimum(len_r, 64)]
    if (len_r > 64).any():
        k_r = np.floor(np.log2(len_r)).astype(np.int64)
    p_r = (1 << k_r).astype(np.int64)
    ia_r = enc(k_r, t + np.zeros_like(rr))
    ib_r = enc(k_r, r_right - p_r)

    flat = np.concatenate([ia_l, ib_l, ia_r, ib_r], axis=1)
    assert flat.min() >= 0 and flat.max() < 2 * NLEV * 128
    return flat


def _wrap_idxs(flat):
    n = flat.shape[0]
    blk = flat.reshape(n // 16, 16).T
    return np.tile(blk, (8, 1))


def kernel(feat: np.ndarray, reg: np.ndarray) -> np.ndarray:
    global LAST_RESULT
    feat = np.ascontiguousarray(feat, dtype=np.float32)
    reg = np.ascontiguousarray(reg, dtype=np.float32)
    assert feat.shape == (B, C, T) and reg.shape == (B, T, 2)

    feat16 = feat.astype(np.float16)
    featT = np.ascontiguousarray(feat16.transpose(0, 2, 1))
    flat = _host_indices(reg)
    gidx = np.stack([_wrap_idxs(flat[b].astype(np.int16)) for b in range(B)])

    nc = _build_graph()
    in_maps = []
    for i in range(N_CORES):
        sl = slice(i * BPC, (i + 1) * BPC)
        in_maps.append({
            "feat16": np.ascontiguousarray(feat16[sl]),
            "featT": np.ascontiguousarray(featT[sl]),
            "gidx": np.ascontiguousarray(gidx[sl]),
        })

    res = run_bass_kernel_spmd(nc, in_maps, list(range(N_CORES)))
    LAST_RESULT = res
    out16 = np.concatenate([res.results[i]["out"] for i in range(N_CORES)],
                           axis=0)
    return out16.astype(np.float32)


# revision 50
# speedup vs baseline: 1.1288x; 1.1288x over previous
"""Trainium2 Bass kernel for variable-window left/right max pooling.

out[b, c, t] = max(feat[b, c, max(t-L,0) : t+1]) + max(feat[b, c, t : min(t+R,T)])
with L = max(0, round(reg[b,t,0])), R = clip(round(reg[b,t,1]), 1, T).

Strategy (2 batches/core, pure data parallel over 8 cores, fp16 on device):
  - range-max sparse table st_k[c, x] = max(feat[c, x:x+2^k]), levels 0..5
    (max window len 33), built in [c%128, lev, cb, t(+pad)] layout with 5
    shifted full-width tensor_tensor(max) DVE ops (t-pad columns absorb the
    shifted reads; memset once).
  - levels 1..5 are stored to DRAM scratch (2 chunks) and XBAR-transpose
    loaded into the token table ttab[t%128, t//128, lev, c] (4 merged-level
    transposes per batch, split across the SP and ACT HWDGE sequencers).
    Level 0 comes from a host-transposed featT upload.
  - window maxes: max of 2 table entries; flat indices
    (t//128)*(6*128) + lev*128 + t%128 precomputed host-side from reg;
    4 transposing SBUF-source row-gathers (2KB rows, 256 idx each) per batch
    emit terms in [c%128, cb, t] layout.
  - 2 max + 1 add on DVE; affine store.  Raw Block mode, explicit semaphores.
"""

import sys
import types

import numpy as np


def _install_profile_shim():
    if "antenv.axon_hooks" in sys.modules:
        return
    try:
        hooks = types.ModuleType("antenv.axon_hooks")
        hooks._hook = None
        hooks.set_axon_ntff_profile_hook = lambda h: setattr(hooks, "_hook", h)
        hooks.get_axon_ntff_profile_hook = lambda: hooks._hook
        sys.modules["antenv.axon_hooks"] = hooks
        import antenv

        antenv.axon_hooks = hooks
        from trn_agent_boot.trn_boot import _ntff_profile_via_ctypes

        hooks.set_axon_ntff_profile_hook(
            _ntff_profile_via_ctypes("/opt/axon/libaxon_pjrt.so")
        )
    except Exception:
        pass


_install_profile_shim()

import concourse.bacc as bacc
import concourse.bass as bass
import concourse.mybir as mybir
from concourse.bass_utils import run_bass_kernel_spmd

B, C, T = 16, 1024, 256
N_CORES = 8
BPC = B // N_CORES
NLEV = 6  # sparse-table levels 0..5
CB = C // 128
NQ = 3 * T  # gathered terms: ia_l, ib_l, ib_r
TP = T + 32  # padded t pitch for full-width shifted builds

_LOG2 = np.zeros(65, dtype=np.int32)
for _n in range(1, 65):
    _LOG2[_n] = _n.bit_length() - 1

_CACHE = {}
LAST_RESULT = None


def _build_graph():
    if "nc" in _CACHE:
        return _CACHE["nc"]

    nc = bacc.Bacc("TRN2", target_bir_lowering=False, debug=False,
                   num_devices=N_CORES)
    f16 = mybir.dt.float16
    i16 = mybir.dt.int16

    feat_ext = nc.dram_tensor("feat16", [BPC, C, T], f16,
                              kind="ExternalInput").ap()
    featT_ext = nc.dram_tensor("featT", [BPC, T, C], f16,
                               kind="ExternalInput").ap()
    gidx_ext = nc.dram_tensor("gidx", [BPC, 128, NQ // 16], i16,
                              kind="ExternalInput").ap()
    kmsk_ext = nc.dram_tensor("kmsk", [BPC, 128, NLEV - 1, T],
                              mybir.dt.uint8, kind="ExternalInput").ap()
    out_ext = nc.dram_tensor("out", [BPC, C, T], f16,
                             kind="ExternalOutput").ap()

    feat_ct = [nc.alloc_sbuf_tensor(f"feat_ct{b}", [128, CB, TP], f16).ap()
               for b in range(BPC)]
    ctab = [nc.alloc_sbuf_tensor(f"ctab{b}", [128, NLEV - 1, CB, TP],
                                 f16).ap() for b in range(BPC)]
    # token table [t%128, rank=lev*2 + t//128, c]; flat idx = lev*T + t
    ttab = [nc.alloc_sbuf_tensor(f"ttab{b}", [128, 2 * NLEV, C], f16).ap()
            for b in range(BPC)]
    gidx = [nc.alloc_sbuf_tensor(f"gidx{b}", [128, NQ // 16], i16).ap()
            for b in range(BPC)]
    gout = [nc.alloc_sbuf_tensor(f"gout{b}", [128, 3, CB, T], f16).ap()
            for b in range(BPC)]
    msk = [nc.alloc_sbuf_tensor(f"msk{b}", [128, NLEV - 1, T],
                                mybir.dt.uint8).ap() for b in range(BPC)]
    racc_a = [nc.alloc_sbuf_tensor(f"racc_a{b}", [128, CB, TP], f16).ap()
              for b in range(BPC)]

    lbuf = [nc.alloc_sbuf_tensor(f"lbuf{b}", [128, CB, T], f16).ap()
            for b in range(BPC)]
    obuf = [nc.alloc_sbuf_tensor(f"obuf{b}", [128, CB, T], f16).ap()
            for b in range(BPC)]
    flbuf = [nc.alloc_sbuf_tensor(f"flbuf{b}", [128, 2 * NLEV, 4], f16).ap()
             for b in range(BPC)]
    scratch = [nc.dram_tensor(f"scratch{b}", [NLEV - 1, C, T], f16).ap()
               for b in range(BPC)]

    with nc.Block() as block:
        s_inf = [nc.alloc_semaphore(f"s_inf{b}") for b in range(BPC)]
        s_int = [nc.alloc_semaphore(f"s_int{b}") for b in range(BPC)]
        s_ing = [nc.alloc_semaphore(f"s_ing{b}") for b in range(BPC)]
        s_bld = [nc.alloc_semaphore(f"s_bld{b}") for b in range(BPC)]
        s_sc = [[nc.alloc_semaphore(f"s_sc{b}_{k}") for k in range(NLEV - 1)]
                for b in range(BPC)]
        s_tt = [nc.alloc_semaphore(f"s_tt{b}") for b in range(BPC)]
        s_g = [nc.alloc_semaphore(f"s_g{b}") for b in range(BPC)]
        s_cmb = [nc.alloc_semaphore(f"s_cmb{b}") for b in range(BPC)]
        s_out = [nc.alloc_semaphore(f"s_out{b}") for b in range(BPC)]
        s_fl = [nc.alloc_semaphore(f"s_fl{b}") for b in range(BPC)]
        s_inm = [nc.alloc_semaphore(f"s_inm{b}") for b in range(BPC)]
        s_sel = [nc.alloc_semaphore(f"s_sel{b}") for b in range(BPC)]

        def emit_stores(eng, b):
            """Per-level scratch stores; s_bld counts 2 memsets + builds."""
            for k in range(1, NLEV):
                eng.wait_ge(s_bld[b], 2 + k)
                eng.dma_start(
                    out=scratch[b][k - 1].rearrange(
                        "(cb p) t -> p cb t", p=128),
                    in_=ctab[b][:, k - 1, :, 0:T],
                ).then_inc(s_sc[b][k - 1], 16)

        def emit_xbars(eng, b):
            # per-(level, tt) [1024, 128] -> [128, 1024] transposes: the one
            # geometry verified bit-exact on hardware.
            for k in range(1, NLEV):
                eng.wait_ge(s_sc[b][k - 1], 16)
                for tt in range(2):
                    eng.dma_start(
                        out=ttab[b][:, 2 * k + tt, :],
                        in_=scratch[b][k - 1][:, tt * 128:(tt + 1) * 128],
                        transpose=True,
                    ).then_inc(s_tt[b], 16)

        @block.sync
        def _(sync):
            sync.dma_start(
                out=feat_ct[0][:, :, 0:T],
                in_=feat_ext[0].rearrange("(cb p) t -> p cb t", p=128),
            ).then_inc(s_inf[0], 16)
            sync.dma_start(
                out=ttab[0][:, 0:2, :],
                in_=featT_ext[0].rearrange("(tt p) c -> p tt c", p=128),
            ).then_inc(s_int[0], 16)
            sync.dma_start(
                out=feat_ct[1][:, :, 0:T],
                in_=feat_ext[1].rearrange("(cb p) t -> p cb t", p=128),
            ).then_inc(s_inf[1], 16)
            sync.dma_start(
                out=ttab[1][:, 0:2, :],
                in_=featT_ext[1].rearrange("(tt p) c -> p tt c", p=128),
            ).then_inc(s_int[1], 16)
            emit_stores(sync, 0)
            emit_stores(sync, 1)
            for b in range(BPC):
                sync.wait_ge(s_cmb[b], 3)
                sync.dma_start(
                    out=out_ext[b].rearrange("(cb p) t -> p cb t", p=128),
                    in_=obuf[b][:, :, :],
                ).then_inc(s_out[b], 16)
            for b in range(BPC):
                sync.wait_ge(s_out[b], 16)

        @block.scalar
        def _(scalar):
            for b in range(BPC):
                scalar.dma_start(out=gidx[b][:, :],
                                 in_=gidx_ext[b]).then_inc(s_ing[b], 16)
                scalar.dma_start(out=msk[b][:, :, :],
                                 in_=kmsk_ext[b]).then_inc(s_inm[b], 16)
            for b in range(BPC):
                emit_xbars(scalar, b)
                # readback after all of this batch's xbar completions: its
                # own completion implies the xbar RX writes are visible
                # before the Pool gathers read ttab.
                scalar.wait_ge(s_tt[b], 16 * 10)
                scalar.dma_start(
                    out=flbuf[b][:, :, :],
                    in_=ttab[b][:, :, 0:4],
                ).then_inc(s_fl[b], 16)

        @block.vector
        def _(vector):
            # interleaved level builds (full width; pad absorbs the shift;
            # pad memsets run on the Pool engine, +2 per b on s_bld)
            for k in range(1, NLEV):
                s = 1 << (k - 1)
                for b in range(BPC):
                    if k == 1:
                        vector.wait_ge(s_inf[b], 16)
                        vector.wait_ge(s_bld[b], 2)
                        src = feat_ct[b][:, :, :]
                    else:
                        vector.wait_ge(s_bld[b], 2 + k - 1)
                        src = ctab[b][:, k - 2, :, :]
                    vector.tensor_tensor(
                        out=ctab[b][:, k - 1, :, 0:T],
                        in0=src[:, :, 0:T],
                        in1=src[:, :, s:s + T],
                        op=mybir.AluOpType.max,
                    ).then_inc(s_bld[b], 1)
            # ia_r term = st_{k_r[t]}[c, t]: masked-select chain over the
            # [c, t] levels (mask one-hots precomputed host-side; k_r==0
            # falls through to feat). Runs in DVE idle time.
            for b in range(BPC):
                vector.wait_ge(s_bld[b], 2 + NLEV - 1)
                vector.wait_ge(s_inm[b], 16)
                vector.tensor_copy(
                    racc_a[b][:, :, 0:T], feat_ct[b][:, :, 0:T],
                ).then_inc(s_sel[b], 1)
                for k in range(1, NLEV):
                    mk = msk[b][:, k - 1, :]
                    mk_b = bass.AP(mk.tensor, mk.offset,
                                   [list(mk.ap[0]), [0, CB], list(mk.ap[1])])
                    vector.wait_ge(s_sel[b], k)
                    vector.copy_predicated(
                        out=racc_a[b][:, :, 0:T],
                        mask=mk_b,
                        data=ctab[b][:, k - 1, :, 0:T],
                    ).then_inc(s_sel[b], 1)
            # combines
            for b in range(BPC):
                vector.wait_ge(s_g[b], 48)
                vector.tensor_tensor(
                    out=lbuf[b][:, :, :],
                    in0=gout[b][:, 0, :, :], in1=gout[b][:, 1, :, :],
                    op=mybir.AluOpType.max,
                ).then_inc(s_cmb[b], 1)
                vector.wait_ge(s_sel[b], NLEV)
                vector.tensor_tensor(
                    out=obuf[b][:, :, :],
                    in0=gout[b][:, 2, :, :],
                    in1=racc_a[b][:, :, 0:T],
                    op=mybir.AluOpType.max,
                ).then_inc(s_cmb[b], 1)
                vector.wait_ge(s_cmb[b], 2)
                vector.tensor_tensor(
                    out=obuf[b][:, :, :], in0=obuf[b][:, :, :],
                    in1=lbuf[b][:, :, :],
                    op=mybir.AluOpType.add,
                ).then_inc(s_cmb[b], 1)

        @block.gpsimd
        def _(gpsimd):
            for b in range(BPC):
                gpsimd.memset(feat_ct[b][:, :, T:TP], 0.0).then_inc(
                    s_bld[b], 1)
                gpsimd.memset(ctab[b][:, :, :, T:TP], 0.0).then_inc(
                    s_bld[b], 1)
            for b in range(BPC):
                gpsimd.wait_ge(s_ing[b], 16)
                gpsimd.wait_ge(s_int[b], 16)
                gpsimd.wait_ge(s_tt[b], 16 * 10)
                gpsimd.wait_ge(s_fl[b], 16)
                for g in range(3):
                    gpsimd.dma_gather(
                        out_ap=gout[b][:, g, :, :],
                        in_ap=ttab[b].rearrange("p r c -> p (r c)"),
                        idxs_ap=gidx[b][:, g * T // 16:(g + 1) * T // 16],
                        num_idxs=T,
                        num_idxs_reg=T,
                        elem_size=C,
                        transpose=True,
                        queue_num=0,
                        sbuf_tokens_per_rank=128,
                        sbuf_free_dim_per_rank=C * 2,
                    ).then_inc(s_g[b], 16)

    nc.compile()
    _CACHE["nc"] = nc
    return nc


def _host_indices(reg):
    """Flat gather indices [B, 4*T]:
    idx(level, x) = (x//128)*(NLEV*128) + level*128 + (x%128).
    Term order: I[term*T + t] = [left_a, left_b, right_a, right_b]."""
    t = np.arange(T, dtype=np.int64)[None, :]

    def enc(k, x):
        return (2 * k + x // 128) * 128 + (x % 128)

    rl = np.maximum(np.round(reg[:, :, 0]).astype(np.int64), 0)
    l_left = np.maximum(t - rl, 0)
    len_l = t + 1 - l_left
    k_l = _LOG2[np.minimum(len_l, 64)]
    if (len_l > 64).any():
        k_l = np.floor(np.log2(len_l)).astype(np.int64)
    p_l = (1 << k_l).astype(np.int64)
    ia_l = enc(k_l, l_left)
    ib_l = enc(k_l, t + 1 - p_l)

    rr = np.clip(np.round(reg[:, :, 1]).astype(np.int64), 1, T)
    r_right = np.minimum(t + rr, T)
    len_r = r_right - t
    k_r = _LOG2[np.minimum(len_r, 64)]
    if (len_r > 64).any():
        k_r = np.floor(np.log2(len_r)).astype(np.int64)
    p_r = (1 << k_r).astype(np.int64)
    ia_r = enc(k_r, t + np.zeros_like(rr))
    ib_r = enc(k_r, r_right - p_r)

    flat = np.concatenate([ia_l, ib_l, ib_r], axis=1)
    assert flat.min() >= 0 and flat.max() < 2 * NLEV * 128
    return flat, k_r


def _wrap_idxs(flat):
    n = flat.shape[0]
    blk = flat.reshape(n // 16, 16).T
    return np.tile(blk, (8, 1))


def kernel(feat: np.ndarray, reg: np.ndarray) -> np.ndarray:
    global LAST_RESULT
    feat = np.ascontiguousarray(feat, dtype=np.float32)
    reg = np.ascontiguousarray(reg, dtype=np.float32)
    assert feat.shape == (B, C, T) and reg.shape == (B, T, 2)

    feat16 = feat.astype(np.float16)
    featT = np.ascontiguousarray(feat16.transpose(0, 2, 1))
    flat, k_r = _host_indices(reg)
    gidx = np.stack([_wrap_idxs(flat[b].astype(np.int16)) for b in range(B)])
    # one-hot level masks for the ia_r select chain, replicated over 128
    # partitions: kmsk[b, :, k-1, t] = (k_r[b, t] == k)
    km = np.stack([(k_r == k).astype(np.uint8) for k in range(1, NLEV)],
                  axis=1)  # [B, 5, T]
    kmsk = np.ascontiguousarray(
        np.broadcast_to(km[:, None, :, :], (B, 128, NLEV - 1, T)))

    nc = _build_graph()
    in_maps = []
    for i in range(N_CORES):
        sl = slice(i * BPC, (i + 1) * BPC)
        in_maps.append({
            "feat16": np.ascontiguousarray(feat16[sl]),
            "featT": np.ascontiguousarray(featT[sl]),
            "gidx": np.ascontiguousarray(gidx[sl]),
            "kmsk": np.ascontiguousarray(kmsk[sl]),
        })

    res = run_bass_kernel_spmd(nc, in_maps, list(range(N_CORES)))
    LAST_RESULT = res
    out16 = np.concatenate([res.results[i]["out"] for i in range(N_CORES)],
                           axis=0)
    return out16.astype(np.float32)


# revision 52
# speedup vs baseline: 1.1576x; 1.0255x over previous
"""Trainium2 Bass kernel for variable-window left/right max pooling.

out[b, c, t] = max(feat[b, c, max(t-L,0) : t+1]) + max(feat[b, c, t : min(t+R,T)])
with L = max(0, round(reg[b,t,0])), R = clip(round(reg[b,t,1]), 1, T).

Strategy (2 batches/core, pure data parallel over 8 cores, fp16 on device):
  - range-max sparse table st_k[c, x] = max(feat[c, x:x+2^k]), levels 0..5
    (max window len 33), built in [c%128, lev, cb, t(+pad)] layout with 5
    shifted full-width tensor_tensor(max) DVE ops (t-pad columns absorb the
    shifted reads; memset once).
  - levels 1..5 are stored to DRAM scratch (2 chunks) and XBAR-transpose
    loaded into the token table ttab[t%128, t//128, lev, c] (4 merged-level
    transposes per batch, split across the SP and ACT HWDGE sequencers).
    Level 0 comes from a host-transposed featT upload.
  - window maxes: max of 2 table entries; flat indices
    (t//128)*(6*128) + lev*128 + t%128 precomputed host-side from reg;
    4 transposing SBUF-source row-gathers (2KB rows, 256 idx each) per batch
    emit terms in [c%128, cb, t] layout.
  - 2 max + 1 add on DVE; affine store.  Raw Block mode, explicit semaphores.
"""

import sys
import types

import numpy as np


def _install_profile_shim():
    if "antenv.axon_hooks" in sys.modules:
        return
    try:
        hooks = types.ModuleType("antenv.axon_hooks")
        hooks._hook = None
        hooks.set_axon_ntff_profile_hook = lambda h: setattr(hooks, "_hook", h)
        hooks.get_axon_ntff_profile_hook = lambda: hooks._hook
        sys.modules["antenv.axon_hooks"] = hooks
        import antenv

        antenv.axon_hooks = hooks
        from trn_agent_boot.trn_boot import _ntff_profile_via_ctypes

        hooks.set_axon_ntff_profile_hook(
            _ntff_profile_via_ctypes("/opt/axon/libaxon_pjrt.so")
        )
    except Exception:
        pass


_install_profile_shim()

import concourse.bacc as bacc
import concourse.bass as bass
import concourse.mybir as mybir
from concourse.bass_utils import run_bass_kernel_spmd

B, C, T = 16, 1024, 256
N_CORES = 8
BPC = B // N_CORES
NLEV = 6  # sparse-table levels 0..5
CB = C // 128
NQ = 2 * T  # gathered terms: ia_l, ib_r
TP = T + 32  # feat pitch: end pad for build shifts
FP = 32      # ctab front pad (absorbs negative ib_l shifted reads)
TP2 = FP + T + 32  # ctab pitch: front + end pads

_LOG2 = np.zeros(65, dtype=np.int32)
for _n in range(1, 65):
    _LOG2[_n] = _n.bit_length() - 1

_CACHE = {}
LAST_RESULT = None


def _build_graph():
    if "nc" in _CACHE:
        return _CACHE["nc"]

    nc = bacc.Bacc("TRN2", target_bir_lowering=False, debug=False,
                   num_devices=N_CORES)
    f16 = mybir.dt.float16
    i16 = mybir.dt.int16

    feat_ext = nc.dram_tensor("feat16", [BPC, C, T], f16,
                              kind="ExternalInput").ap()
    featT_ext = nc.dram_tensor("featT", [BPC, T, C], f16,
                               kind="ExternalInput").ap()
    gidx_ext = nc.dram_tensor("gidx", [BPC, 128, NQ // 16], i16,
                              kind="ExternalInput").ap()
    kmsk_ext = nc.dram_tensor("kmsk", [BPC, 128, 2, NLEV - 1, T],
                              mybir.dt.uint8, kind="ExternalInput").ap()
    out_ext = nc.dram_tensor("out", [BPC, C, T], f16,
                             kind="ExternalOutput").ap()

    feat_ct = [nc.alloc_sbuf_tensor(f"feat_ct{b}", [128, CB, TP], f16).ap()
               for b in range(BPC)]
    ctab = [nc.alloc_sbuf_tensor(f"ctab{b}", [128, NLEV - 1, CB, TP2],
                                 f16).ap() for b in range(BPC)]
    # token table [t%128, rank=lev*2 + t//128, c]; flat idx = lev*T + t
    ttab = [nc.alloc_sbuf_tensor(f"ttab{b}", [128, 2 * NLEV, C], f16).ap()
            for b in range(BPC)]
    gidx = [nc.alloc_sbuf_tensor(f"gidx{b}", [128, NQ // 16], i16).ap()
            for b in range(BPC)]
    gout = [nc.alloc_sbuf_tensor(f"gout{b}", [128, 2, CB, T], f16).ap()
            for b in range(BPC)]
    msk = [nc.alloc_sbuf_tensor(f"msk{b}", [128, 2, NLEV - 1, T],
                                mybir.dt.uint8).ap() for b in range(BPC)]
    racc_a = [nc.alloc_sbuf_tensor(f"racc_a{b}", [128, CB, TP], f16).ap()
              for b in range(BPC)]
    lacc_a = [nc.alloc_sbuf_tensor(f"lacc_a{b}", [128, CB, TP], f16).ap()
              for b in range(BPC)]

    lbuf = [nc.alloc_sbuf_tensor(f"lbuf{b}", [128, CB, T], f16).ap()
            for b in range(BPC)]
    obuf = [nc.alloc_sbuf_tensor(f"obuf{b}", [128, CB, T], f16).ap()
            for b in range(BPC)]
    flbuf = [nc.alloc_sbuf_tensor(f"flbuf{b}", [128, 2 * NLEV, 4], f16).ap()
             for b in range(BPC)]
    scratch = [nc.dram_tensor(f"scratch{b}", [NLEV - 1, C, T], f16).ap()
               for b in range(BPC)]

    with nc.Block() as block:
        s_inf = [nc.alloc_semaphore(f"s_inf{b}") for b in range(BPC)]
        s_int = [nc.alloc_semaphore(f"s_int{b}") for b in range(BPC)]
        s_ing = [nc.alloc_semaphore(f"s_ing{b}") for b in range(BPC)]
        s_bld = [nc.alloc_semaphore(f"s_bld{b}") for b in range(BPC)]
        s_sc = [[nc.alloc_semaphore(f"s_sc{b}_{k}") for k in range(NLEV - 1)]
                for b in range(BPC)]
        s_tt = [nc.alloc_semaphore(f"s_tt{b}") for b in range(BPC)]
        s_g = [nc.alloc_semaphore(f"s_g{b}") for b in range(BPC)]
        s_cmb = [nc.alloc_semaphore(f"s_cmb{b}") for b in range(BPC)]
        s_out = [nc.alloc_semaphore(f"s_out{b}") for b in range(BPC)]
        s_fl = [nc.alloc_semaphore(f"s_fl{b}") for b in range(BPC)]
        s_inm = [nc.alloc_semaphore(f"s_inm{b}") for b in range(BPC)]
        s_sel = [nc.alloc_semaphore(f"s_sel{b}") for b in range(BPC)]
        s_sell = [nc.alloc_semaphore(f"s_sell{b}") for b in range(BPC)]

        def emit_stores(eng, b):
            """Per-level scratch stores; s_bld counts 2 memsets + builds."""
            for k in range(1, NLEV):
                eng.wait_ge(s_bld[b], 3 + k)
                eng.dma_start(
                    out=scratch[b][k - 1].rearrange(
                        "(cb p) t -> p cb t", p=128),
                    in_=ctab[b][:, k - 1, :, FP:FP + T],
                ).then_inc(s_sc[b][k - 1], 16)

        def emit_xbars(eng, b):
            # per-(level, tt) [1024, 128] -> [128, 1024] transposes: the one
            # geometry verified bit-exact on hardware.
            for k in range(1, NLEV):
                eng.wait_ge(s_sc[b][k - 1], 16)
                for tt in range(2):
                    eng.dma_start(
                        out=ttab[b][:, 2 * k + tt, :],
                        in_=scratch[b][k - 1][:, tt * 128:(tt + 1) * 128],
                        transpose=True,
                    ).then_inc(s_tt[b], 16)

        @block.sync
        def _(sync):
            sync.dma_start(
                out=feat_ct[0][:, :, 0:T],
                in_=feat_ext[0].rearrange("(cb p) t -> p cb t", p=128),
            ).then_inc(s_inf[0], 16)
            sync.dma_start(
                out=ttab[0][:, 0:2, :],
                in_=featT_ext[0].rearrange("(tt p) c -> p tt c", p=128),
            ).then_inc(s_int[0], 16)
            sync.dma_start(
                out=feat_ct[1][:, :, 0:T],
                in_=feat_ext[1].rearrange("(cb p) t -> p cb t", p=128),
            ).then_inc(s_inf[1], 16)
            sync.dma_start(
                out=ttab[1][:, 0:2, :],
                in_=featT_ext[1].rearrange("(tt p) c -> p tt c", p=128),
            ).then_inc(s_int[1], 16)
            emit_stores(sync, 0)
            emit_stores(sync, 1)
            for b in range(BPC):
                sync.wait_ge(s_cmb[b], 3)
                sync.dma_start(
                    out=out_ext[b].rearrange("(cb p) t -> p cb t", p=128),
                    in_=obuf[b][:, :, :],
                ).then_inc(s_out[b], 16)
            for b in range(BPC):
                sync.wait_ge(s_out[b], 16)

        @block.scalar
        def _(scalar):
            for b in range(BPC):
                scalar.dma_start(out=gidx[b][:, :],
                                 in_=gidx_ext[b]).then_inc(s_ing[b], 16)
                scalar.dma_start(out=msk[b][:, :, :],
                                 in_=kmsk_ext[b]).then_inc(s_inm[b], 16)
            for b in range(BPC):
                emit_xbars(scalar, b)
                # readback after all of this batch's xbar completions: its
                # own completion implies the xbar RX writes are visible
                # before the Pool gathers read ttab.
                scalar.wait_ge(s_tt[b], 16 * 10)
                scalar.dma_start(
                    out=flbuf[b][:, :, :],
                    in_=ttab[b][:, :, 0:4],
                ).then_inc(s_fl[b], 16)

        @block.vector
        def _(vector):
            # interleaved level builds (full width; pad absorbs the shift;
            # pad memsets run on the Pool engine, +2 per b on s_bld)
            for k in range(1, NLEV):
                s = 1 << (k - 1)
                for b in range(BPC):
                    if k == 1:
                        vector.wait_ge(s_inf[b], 16)
                        vector.wait_ge(s_bld[b], 3)
                        src = feat_ct[b][:, :, 0:TP]
                        o = 0
                    else:
                        vector.wait_ge(s_bld[b], 3 + k - 1)
                        src = ctab[b][:, k - 2, :, :]
                        o = FP
                    vector.tensor_tensor(
                        out=ctab[b][:, k - 1, :, FP:FP + T],
                        in0=src[:, :, o:o + T],
                        in1=src[:, :, o + s:o + s + T],
                        op=mybir.AluOpType.max,
                    ).then_inc(s_bld[b], 1)
            # ia_r term = st_{k_r[t]}[c, t]: masked-select chain over the
            # [c, t] levels (mask one-hots precomputed host-side; k_r==0
            # falls through to feat). Runs in DVE idle time.
            for b in range(BPC):
                vector.wait_ge(s_bld[b], 3 + NLEV - 1)
                vector.wait_ge(s_inm[b], 16)
                vector.tensor_copy(
                    racc_a[b][:, :, 0:T], feat_ct[b][:, :, 0:T],
                ).then_inc(s_sel[b], 1)
                vector.tensor_copy(
                    lacc_a[b][:, :, 0:T], feat_ct[b][:, :, 0:T],
                ).then_inc(s_sell[b], 1)
                for k in range(1, NLEV):
                    mk = msk[b][:, 1, k - 1, :]
                    mk_b = bass.AP(mk.tensor, mk.offset,
                                   [list(mk.ap[0]), [0, CB], list(mk.ap[1])])
                    vector.wait_ge(s_sel[b], k)
                    vector.copy_predicated(
                        out=racc_a[b][:, :, 0:T],
                        mask=mk_b,
                        data=ctab[b][:, k - 1, :, FP:FP + T],
                    ).then_inc(s_sel[b], 1)
                    # ib_l = st_k[t + 1 - 2^k]: shifted read into front pad
                    ml = msk[b][:, 0, k - 1, :]
                    ml_b = bass.AP(ml.tensor, ml.offset,
                                   [list(ml.ap[0]), [0, CB], list(ml.ap[1])])
                    sh = FP + 1 - (1 << k)
                    vector.wait_ge(s_sell[b], k)
                    vector.copy_predicated(
                        out=lacc_a[b][:, :, 0:T],
                        mask=ml_b,
                        data=ctab[b][:, k - 1, :, sh:sh + T],
                    ).then_inc(s_sell[b], 1)
            # combines
            for b in range(BPC):
                vector.wait_ge(s_g[b], 32)
                vector.wait_ge(s_sell[b], NLEV)
                vector.tensor_tensor(
                    out=lbuf[b][:, :, :],
                    in0=gout[b][:, 0, :, :], in1=lacc_a[b][:, :, 0:T],
                    op=mybir.AluOpType.max,
                ).then_inc(s_cmb[b], 1)
                vector.wait_ge(s_sel[b], NLEV)
                vector.tensor_tensor(
                    out=obuf[b][:, :, :],
                    in0=gout[b][:, 1, :, :],
                    in1=racc_a[b][:, :, 0:T],
                    op=mybir.AluOpType.max,
                ).then_inc(s_cmb[b], 1)
                vector.wait_ge(s_cmb[b], 2)
                vector.tensor_tensor(
                    out=obuf[b][:, :, :], in0=obuf[b][:, :, :],
                    in1=lbuf[b][:, :, :],
                    op=mybir.AluOpType.add,
                ).then_inc(s_cmb[b], 1)

        @block.gpsimd
        def _(gpsimd):
            for b in range(BPC):
                gpsimd.memset(feat_ct[b][:, :, T:TP], 0.0).then_inc(
                    s_bld[b], 1)
                gpsimd.memset(ctab[b][:, :, :, 0:FP], 0.0).then_inc(
                    s_bld[b], 1)
                gpsimd.memset(ctab[b][:, :, :, FP + T:TP2], 0.0).then_inc(
                    s_bld[b], 1)
            for b in range(BPC):
                gpsimd.wait_ge(s_ing[b], 16)
                gpsimd.wait_ge(s_int[b], 16)
                gpsimd.wait_ge(s_tt[b], 16 * 10)
                gpsimd.wait_ge(s_fl[b], 16)
                for g in range(2):
                    gpsimd.dma_gather(
                        out_ap=gout[b][:, g, :, :],
                        in_ap=ttab[b].rearrange("p r c -> p (r c)"),
                        idxs_ap=gidx[b][:, g * T // 16:(g + 1) * T // 16],
                        num_idxs=T,
                        num_idxs_reg=T,
                        elem_size=C,
                        transpose=True,
                        queue_num=0,
                        sbuf_tokens_per_rank=128,
                        sbuf_free_dim_per_rank=C * 2,
                    ).then_inc(s_g[b], 16)

    nc.compile()
    _CACHE["nc"] = nc
    return nc


def _host_indices(reg):
    """Flat gather indices [B, 4*T]:
    idx(level, x) = (x//128)*(NLEV*128) + level*128 + (x%128).
    Term order: I[term*T + t] = [left_a, left_b, right_a, right_b]."""
    t = np.arange(T, dtype=np.int64)[None, :]

    def enc(k, x):
        return (2 * k + x // 128) * 128 + (x % 128)

    rl = np.maximum(np.round(reg[:, :, 0]).astype(np.int64), 0)
    l_left = np.maximum(t - rl, 0)
    len_l = t + 1 - l_left
    k_l = _LOG2[np.minimum(len_l, 64)]
    if (len_l > 64).any():
        k_l = np.floor(np.log2(len_l)).astype(np.int64)
    p_l = (1 << k_l).astype(np.int64)
    ia_l = enc(k_l, l_left)
    ib_l = enc(k_l, t + 1 - p_l)

    rr = np.clip(np.round(reg[:, :, 1]).astype(np.int64), 1, T)
    r_right = np.minimum(t + rr, T)
    len_r = r_right - t
    k_r = _LOG2[np.minimum(len_r, 64)]
    if (len_r > 64).any():
        k_r = np.floor(np.log2(len_r)).astype(np.int64)
    p_r = (1 << k_r).astype(np.int64)
    ia_r = enc(k_r, t + np.zeros_like(rr))
    ib_r = enc(k_r, r_right - p_r)

    flat = np.concatenate([ia_l, ib_r], axis=1)
    assert flat.min() >= 0 and flat.max() < 2 * NLEV * 128
    return flat, k_l, k_r


def _wrap_idxs(flat):
    n = flat.shape[0]
    blk = flat.reshape(n // 16, 16).T
    return np.tile(blk, (8, 1))


def kernel(feat: np.ndarray, reg: np.ndarray) -> np.ndarray:
    global LAST_RESULT
    feat = np.ascontiguousarray(feat, dtype=np.float32)
    reg = np.ascontiguousarray(reg, dtype=np.float32)
    assert feat.shape == (B, C, T) and reg.shape == (B, T, 2)

    feat16 = feat.astype(np.float16)
    featT = np.ascontiguousarray(feat16.transpose(0, 2, 1))
    flat, k_l, k_r = _host_indices(reg)
    gidx = np.stack([_wrap_idxs(flat[b].astype(np.int16)) for b in range(B)])
    # one-hot level masks for the ib_l / ia_r select chains, replicated over
    # 128 partitions: kmsk[b, :, 0/1, k-1, t] = (k_{l/r}[b, t] == k)
    km = np.stack([np.stack([(kk == k).astype(np.uint8)
                             for k in range(1, NLEV)], axis=1)
                   for kk in (k_l, k_r)], axis=1)  # [B, 2, 5, T]
    kmsk = np.ascontiguousarray(
        np.broadcast_to(km[:, None], (B, 128, 2, NLEV - 1, T)))

    nc = _build_graph()
    in_maps = []
    for i in range(N_CORES):
        sl = slice(i * BPC, (i + 1) * BPC)
        in_maps.append({
            "feat16": np.ascontiguousarray(feat16[sl]),
            "featT": np.ascontiguousarray(featT[sl]),
            "gidx": np.ascontiguousarray(gidx[sl]),
            "kmsk": np.ascontiguousarray(kmsk[sl]),
        })

    res = run_bass_kernel_spmd(nc, in_maps, list(range(N_CORES)))
    LAST_RESULT = res
    out16 = np.concatenate([res.results[i]["out"] for i in range(N_CORES)],
                           axis=0)
    return out16.astype(np.float32)
